# revision 8
# baseline (speedup 1.0000x reference)
"""CVQNN classifier kernel for 8 Trainium2 NeuronCores.

Math: the whole quantum circuit collapses to a batch-independent affine map
(S, d) on 128-dim phase space.  Per batch row the heavy work is
    m = x @ W2 + d20          (W2 = S[rows, :64].T, shape (64, 20))
    out_k = log1p(m_x[k]^2 + m_p[k]^2 + covc_k)
i.e. a (B,64) @ (64,20) matmul + elementwise tail -> (B,10).  Memory bound.
(The reference's relu is a provable no-op: covc >= 0 for symplectic S.)

The rel-err budget (2e-2) admits pure bf16 x and W (measured 2.9e-3), so
unlike the previous hi/lo-split version this one moves HALF the input
bytes and also returns bf16 outputs (host upcasts).

Device layout (per core, R = 125952 rows, 2 rows per xstack column):
  - xstack (128, 62976) bf16: partitions 0..63 = features of row-group A,
    64..127 = features of row-group B (consecutive 128*w-row chunks of
    each super-block).  Block-diagonal weights wcat (128, 40) =
    [[Wh, 0], [0, Wh]] make one 128x128-stationary matmul produce
    40 psum cols = 10 classes x {x,p} x {A,B} for 256 rows.
  - per super-block (48 j-blocks = 12288 rows): 1 input DMA [128, 6144]
    (12 KB per-partition lines), 4 bank-preload matmuls (stationary =
    ones/128, moving = d-pattern, N=480) fold the +d into PSUM, then 48
    accumulating matmuls (start=False).  No DVE bias-add needed.
  - tail: ACT square (psum -> sbuf), DVE pair-add + cov-add, ACT ln(1+.)
    straight to bf16.  ln(s-1) is emitted after square(s) so the in-order
    ACT queue never stalls on the DVE chain.
  - all DMA on the two HWDGE rings (input on sync/SP, output on
    scalar/ACT) - no SWDGE descriptor-ring traffic at all.
"""

import ml_dtypes
import numpy as np

import concourse.bacc as bacc
import concourse.mybir as mybir
import concourse.tile as tile
from concourse.bass_utils import run_bass_kernel_spmd

N = 64          # wires
OUT = 10        # measured wires / classes
NCORES = 8
JBLK = 48                  # j-blocks per full super-block (4 psum banks)
# j-block = 256 rows (2 row-groups x 128).  10 full super-blocks + one
# 12-j tail: 492 j-blocks, 125952 rows/core, 0.76% padding.
WIDTHS = [JBLK] * 10 + [12]
JTOT = sum(WIDTHS)         # 492
R = 256 * JTOT             # per-core rows = 125952
CC = 128 * JTOT            # per-core xstack cols = 62976
B_PAD = R * NCORES         # 1007616
F32 = mybir.dt.float32
BF16 = mybir.dt.bfloat16
NPBF16 = ml_dtypes.bfloat16


# ---------------------------------------------------------------- host math
def _bs_pass(n, start, int_params):
    i = np.arange(start, n - 1, 2)
    j = i + 1
    theta = int_params[3 * i]
    phi = int_params[3 * i + 1]
    ct, st = np.cos(theta), np.sin(theta)
    cp, sp = np.cos(phi), np.sin(phi)
    S = np.eye(2 * n)
    S[i, i] = ct
    S[i, j] = -cp * st
    S[i, n + j] = -sp * st
    S[j, i] = cp * st
    S[j, j] = ct
    S[j, n + i] = -sp * st
    S[n + i, j] = sp * st
    S[n + i, n + i] = ct
    S[n + i, n + j] = -cp * st
    S[n + j, i] = sp * st
    S[n + j, n + i] = cp * st
    S[n + j, n + j] = ct
    return S


def _layer_symplectic(n, int1, squeezes, int2):
    M = _bs_pass(n, 0, int1)
    M = _bs_pass(n, 1, int1) @ M
    c = np.concatenate([np.cos(int1[2::3]), np.ones(1)])
    s = np.concatenate([np.sin(int1[2::3]), np.zeros(1)])
    Rm = np.block([[np.diag(c), np.diag(-s)], [np.diag(s), np.diag(c)]])
    Sq = np.diag(np.concatenate([np.exp(-squeezes), np.exp(squeezes)]))
    M = Sq @ (Rm @ M)
    M = _bs_pass(n, 0, int2) @ M
    M = _bs_pass(n, 1, int2) @ M
    return M


def _affine_map(layers):
    n = N
    S = np.eye(2 * n)
    d = np.zeros(2 * n)
    for int1, sq, int2, disp in layers:
        M = _layer_symplectic(n, int1, sq, int2)
        S = M @ S
        d = M @ d
        d[:n] += 2.0 * disp
    return S, d


def _device_constants(layers):
    S, d = _affine_map(layers)
    w = np.arange(OUT)
    rows = np.concatenate([w, N + w])
    cov = S @ S.T
    cov_term = cov[w, w] + cov[N + w, N + w]            # (10,)
    W2 = S[rows, :N].T.astype(np.float32)               # (64, 20)
    d20 = (d[rows] / 2.0).astype(np.float32)            # (20,)
    covc = np.maximum(cov_term / 4.0 - 0.5, 0.0).astype(np.float32)

    Wh = W2.astype(NPBF16)
    wcat = np.zeros((128, 40), NPBF16)                  # [[Wh, 0], [0, Wh]]
    wcat[0:64, 0:20] = Wh
    wcat[64:128, 20:40] = Wh

    ones = np.ones((128, 128), NPBF16)
    # bank preload pattern: 12 slots x [d20 | d20]; moving operand is
    # d/128 so the 128-partition ones-contraction reconstitutes d exactly
    dpat = np.ascontiguousarray(np.broadcast_to(
        np.tile(d20, 24) / 128.0, (128, 480))).astype(NPBF16)
    cconst = np.ascontiguousarray(np.broadcast_to(
        np.tile(covc, 2 * JBLK), (128, 20 * JBLK))).astype(np.float32)
    return wcat, ones, dpat, cconst


# ---------------------------------------------------------------- bass build
def build_nc(widths=None):
    widths = widths or WIDTHS
    jtot = sum(widths)
    nc = bacc.Bacc("TRN2", target_bir_lowering=False)
    OC = 20 * JBLK                             # out cols per full sb (960)
    xs = nc.dram_tensor("xs", (128, 128 * jtot), BF16, kind="ExternalInput")
    wst = nc.dram_tensor("wcat", (128, 40), BF16, kind="ExternalInput")
    onest = nc.dram_tensor("ones", (128, 128), BF16, kind="ExternalInput")
    dpatt = nc.dram_tensor("dpat", (128, 480), BF16, kind="ExternalInput")
    ccon = nc.dram_tensor("covconst", (128, OC), F32, kind="ExternalInput")
    out = nc.dram_tensor("out", (128, 20 * jtot), BF16, kind="ExternalOutput")

    Square = mybir.ActivationFunctionType.Square
    Ln = mybir.ActivationFunctionType.Ln

    with tile.TileContext(nc) as tc:
        with (
            tc.tile_pool(name="const", bufs=1) as cpool,
            tc.tile_pool(name="xin", bufs=4) as xpool,
            tc.tile_pool(name="mid", bufs=3) as mpool,
            tc.tile_pool(name="ob", bufs=3) as opool,
            tc.tile_pool(name="ps", bufs=2, space="PSUM") as pspool,
        ):
            # consts gate the first matmuls: load on the input (sync) ring
            w_t = cpool.tile([128, 40], BF16)
            ones_t = cpool.tile([128, 128], BF16)
            dpat_t = cpool.tile([128, 480], BF16)
            c_t = cpool.tile([128, OC], F32)

            def load_consts():
                nc.sync.dma_start(w_t[:], wst[:])
                nc.sync.dma_start(ones_t[:], onest[:])
                nc.sync.dma_start(dpat_t[:], dpatt[:])
                nc.scalar.dma_start(c_t[:], ccon[:])

            pending = []

            def flush_pending(keep=0):
                # ln(s-2) runs here, two super-blocks behind: its input (v)
                # is long since ready, so the in-order ACT queue never waits
                # on the DVE chain.  Output DMA issues from gpsimd (SWDGE)
                # to keep the ACT queue free for square/ln.
                while len(pending) > keep:
                    v, oc, ob = pending.pop(0)
                    o = opool.tile([128, oc], BF16, tag="o")
                    nc.scalar.activation(o[:], v[:], Ln, bias=1.0)
                    nc.gpsimd.dma_start(out[:, ob:ob + oc], o[:])

            def emit_sb(col_base, jblk, in_chunks):
                wc, oc, nbank = 40 * jblk, 20 * jblk, jblk // 12
                w = 128 * jblk
                tin = xpool.tile([128, w], BF16, tag="tin")
                q = w // in_chunks
                for c4 in range(in_chunks):
                    nc.sync.dma_start(
                        tin[:, c4 * q:(c4 + 1) * q],
                        xs[:, col_base + c4 * q:col_base + (c4 + 1) * q])
                    if col_base == 0 and c4 == 0:
                        # consts ride the sync ring right behind the first
                        # input chunk: stream starts earlier, matmuls still
                        # gated only ~1us later
                        load_consts()

                # psum: 12 j-blocks in the first 480 cols of each bank;
                # bank preload folds +d into the accumulation
                ps = pspool.tile([128, nbank, 512], F32, tag="ps")
                for t in range(nbank):
                    nc.tensor.matmul(ps[:, t, 0:480], ones_t[:], dpat_t[:],
                                     start=True, stop=False)
                for j in range(jblk):
                    nc.tensor.matmul(
                        ps[:, j // 12, 40 * (j % 12):40 * (j % 12) + 40],
                        tin[:, 128 * j:128 * j + 128], w_t[:],
                        start=False, stop=True,
                    )

                # software-pipelined: ln+store from two sbs back go BEFORE
                # our square on the ACT queue
                flush_pending(keep=1)

                sq = mpool.tile([128, wc], F32, tag="sq")
                sqv = sq[:].rearrange("p (t c) -> p t c", t=nbank)
                nc.scalar.activation(sqv, ps[:, :, 0:480], Square)

                sq2 = sq[:].rearrange("p (g xp k) -> p g xp k", xp=2, k=10)
                s = mpool.tile([128, oc], F32, tag="s")
                sv = s[:].rearrange("p (g k) -> p g k", k=10)
                nc.vector.tensor_add(sv, sq2[:, :, 0, :], sq2[:, :, 1, :])
                v = mpool.tile([128, oc], F32, tag="v")
                nc.vector.tensor_add(v[:], s[:], c_t[:, 0:oc])
                pending.append((v, oc, (col_base // 128) * 20))

            # first tile's DMA in eighths so compute starts sooner
            col = 0
            for i, wdt in enumerate(widths):
                emit_sb(col, wdt, 8 if i == 0 else 1)
                col += 128 * wdt
            flush_pending()
    nc.compile()
    return nc


# ---------------------------------------------------------------- host glue
def _make_in_maps(x_batch, wcat, ones, dpat, cconst):
    B = x_batch.shape[0]
    xpad = np.zeros((B_PAD, N), np.float32)
    xpad[:B] = x_batch
    xh = xpad.astype(NPBF16)
    in_maps = []
    for c in range(NCORES):
        xc = xh[c * R:(c + 1) * R]
        xstk = np.empty((128, CC), NPBF16)
        # full super-blocks: rows (s, grp, t, f) -> col grp*64+f, s*6144+t
        nfull = (len(WIDTHS) - 1) * 256 * JBLK              # 122880 rows
        ccf = (len(WIDTHS) - 1) * 128 * JBLK                # 61440 cols
        xr = xc[:nfull].reshape(len(WIDTHS) - 1, 2, 128 * JBLK, N)
        xstk[:, :ccf] = xr.transpose(1, 3, 0, 2).reshape(128, ccf)
        xt = xc[nfull:].reshape(2, (R - nfull) // 2, N)
        xstk[:, ccf:] = xt.transpose(0, 2, 1).reshape(128, CC - ccf)
        in_maps.append({"xs": xstk, "wcat": wcat, "ones": ones,
                        "dpat": dpat, "covconst": cconst})
    return in_maps


def _decode_out(results, B):
    full = np.empty((B_PAD, OUT), np.float32)
    nfull = (len(WIDTHS) - 1) * 128 * JBLK * 2
    ocf = (len(WIDTHS) - 1) * 20 * JBLK
    for c in range(NCORES):
        O = results[c]["out"].astype(np.float32)
        Of = O[:, :ocf].reshape(128, len(WIDTHS) - 1, JBLK, 2, OUT)
        full[c * R:c * R + nfull] = (
            Of.transpose(1, 3, 2, 0, 4).reshape(nfull, OUT))
        Ot = O[:, ocf:].reshape(128, WIDTHS[-1], 2, OUT)
        full[c * R + nfull:(c + 1) * R] = (
            Ot.transpose(2, 1, 0, 3).reshape(R - nfull, OUT))
    return full[:B]


_NC_CACHE = {}


def kernel(x_batch, int1_0, squeezes_0, int2_0, disp_0,
           int1_1, squeezes_1, int2_1, disp_1, _trace=False):
    layers = [
        (np.asarray(int1_0, np.float64), np.asarray(squeezes_0, np.float64),
         np.asarray(int2_0, np.float64), np.asarray(disp_0, np.float64)),
        (np.asarray(int1_1, np.float64), np.asarray(squeezes_1, np.float64),
         np.asarray(int2_1, np.float64), np.asarray(disp_1, np.float64)),
    ]
    wcat, ones, dpat, cconst = _device_constants(layers)
    in_maps = _make_in_maps(np.asarray(x_batch, np.float32),
                            wcat, ones, dpat, cconst)

    if "nc" not in _NC_CACHE:
        _NC_CACHE["nc"] = build_nc()
    nc = _NC_CACHE["nc"]

    res = run_bass_kernel_spmd(
        nc, in_maps, core_ids=list(range(NCORES)), trace=_trace
    )
    out = _decode_out(res.results, x_batch.shape[0])
    if _trace:
        return out, res
    return out
